# revision 1
# baseline (speedup 1.0000x reference)
"""Masked softmax attention (B=2,H=16,S=2048,D=64) on 8 trn2 NeuronCores.

Sharding: B*H=32 head-slices, 4 heads per core (pure data parallel),
mask replicated. Per head the device computes scores TRANSPOSED
(S_T[k,q] = K @ Q^T, contract d=64) so that softmax-normalisation and
attn@V need no on-chip transposes:

  E_T[k,q] = exp(S_T[k,q]/8) * keep01_T[k,q]          (ACT exp + DVE mult)
  outT[d,q], denom[q] = vA^T @ E_T  (vA = [V | ones])  (PE, contract k)
  out[d,q] = outT[d,q] * (1/denom[q])                  (DVE recip + mult)

Host packs qT/kT [h,d,S] bf16, vA [h,S,65] bf16, keep-mask-T bf16, and
transposes the [h,d,S] f32 result back to [B,H,S,D].
"""

import os
import sys
from contextlib import ExitStack

import numpy as np

for _p in ("/opt/trn_rl_repo",):
    if _p not in sys.path:
        sys.path.insert(0, _p)

import ml_dtypes  # noqa: E402

import concourse.bass as bass  # noqa: E402
import concourse.mybir as mybir  # noqa: E402
import concourse.tile as tile  # noqa: E402
from concourse import bacc  # noqa: E402
from concourse.bass_utils import run_bass_kernel_spmd  # noqa: E402
from concourse.tile_rust import add_dep_helper  # noqa: E402

B, H, S, D = 2, 16, 2048, 64
NCORES = 8
HPC = (B * H) // NCORES  # heads per core
P = 128
SKT = S // P  # 16 sk tiles of 128 rows
SQW = 512  # matmul moving-dim width
SQB = S // SQW  # 4
HALF = S // 2

BF16 = mybir.dt.bfloat16
F32 = mybir.dt.float32
NPBF16 = ml_dtypes.bfloat16

LAST_EXEC_TIME_NS = None
LAST_RESULTS = None
VARIANT = "full"  # "full" | "nodve" (skip mask mult) | "noact" (skip exp too)
CHAIN_PE = False
DEDUP_LDW = False
EVICT = False
WARMUP = 0  # matmuls of N=512 issued back-to-back at start to HAM-warm the PE
HOSTNORM = True  # cores return numerator+denominator; host divides on unshard
OBUF = 8  # ostage pool depth
BIGMASK = True  # one [P,S] mask mult per tile instead of two halves
SCFULL = False  # single [P,S] scores tile + one exp call per tile
PVLAST = False  # emit exp/mask(t) before PV(t-1) in program order
EVICT_ACT = False  # hostnorm eviction copy on ScalarE instead of DVE


def _emit(tc, qT_d, kT_d, vA_d, mT_d, outT_d, loop_n=0, hoist=False):
    if loop_n and not hoist:
        # timing-only variant: run the whole body loop_n times in-NEFF so
        # per-iteration device time can be measured without NTFF profiling
        with tc.For_i(
            0, loop_n, 1, hint_engines=(mybir.EngineType.PE,)
        ):
            _emit_body(tc, qT_d, kT_d, vA_d, mT_d, outT_d)
    else:
        _emit_body(
            tc, qT_d, kT_d, vA_d, mT_d, outT_d, loop_n=loop_n, hoist=hoist
        )


def _emit_body(tc, qT_d, kT_d, vA_d, mT_d, outT_d, loop_n=0, hoist=False):
    nc = tc.nc
    Exp = mybir.ActivationFunctionType.Exp
    with ExitStack() as ctx:
        const = ctx.enter_context(tc.tile_pool(name="const", bufs=1))
        epool = ctx.enter_context(tc.tile_pool(name="epool", bufs=1))
        small = ctx.enter_context(tc.tile_pool(name="small", bufs=2))
        ostage = ctx.enter_context(tc.tile_pool(name="ostage", bufs=OBUF))
        spsum = ctx.enter_context(tc.tile_pool(name="spsum", bufs=1, space="PSUM"))
        opsum = ctx.enter_context(tc.tile_pool(name="opsum", bufs=1, space="PSUM"))
        dpool = ctx.enter_context(tc.tile_pool(name="dpool", bufs=2, space="DRAM"))

        # ---- resident inputs ----
        # head 0's q/k first so PE can start immediately; mask tiles next in
        # consumption order; remaining heads' tensors last
        mask_sb = const.tile([P, SKT, S], BF16)
        qT_sb = const.tile([D, HPC, S], BF16)
        kT_sb = const.tile([D, HPC, S], BF16)
        vA_sb = const.tile([P, HPC, SKT, D + 1], BF16)
        nc.sync.dma_start(kT_sb[:, 0, :], kT_d[0])
        nc.sync.dma_start(qT_sb[:, 0, :], qT_d[0])
        nc.sync.dma_start(
            vA_sb[:, 0, :, :], vA_d[0].rearrange("(c p) e -> p c e", p=P)
        )
        for t in range(SKT):
            nc.sync.dma_start(mask_sb[:, t, :], mT_d[t * P : (t + 1) * P, :])
        for h in range(1, HPC):
            nc.sync.dma_start(qT_sb[:, h, :], qT_d[h])
            nc.sync.dma_start(kT_sb[:, h, :], kT_d[h])
            nc.sync.dma_start(
                vA_sb[:, h, :, :], vA_d[h].rearrange("(c p) e -> p c e", p=P)
            )

        if loop_n and hoist:
            # timing variant: inputs loaded once, compute looped
            with tc.For_i(0, loop_n, 1, hint_engines=(mybir.EngineType.PE,)):
                _compute(tc, ctx, locals())
            return
        _compute(tc, ctx, locals())


def _compute(tc, ctx, env):
    nc = tc.nc
    Exp = mybir.ActivationFunctionType.Exp
    mask_sb = env["mask_sb"]
    qT_sb = env["qT_sb"]
    kT_sb = env["kT_sb"]
    vA_sb = env["vA_sb"]
    epool = env["epool"]
    small = env["small"]
    ostage = env["ostage"]
    spsum = env["spsum"]
    opsum = env["opsum"]
    dpool = env["dpool"]
    outT_d = env["outT_d"]

    prev_mm = [None]

    def mm(*args, **kwargs):
        # optional chaining of PE matmuls in emission order (measured slower
        # on HW than the tile scheduler's interleaving — off by default)
        inst = nc.tensor.matmul(*args, **kwargs)
        if CHAIN_PE and prev_mm[0] is not None:
            add_dep_helper(inst.ins, prev_mm[0].ins, sync=False, reason="pe order")
        prev_mm[0] = inst
        return inst

    if WARMUP:
        # ~4.3us of dense back-to-back matmuls so the PE HAM clock gate
        # opens to 8/8 before the pipelined (bursty) main loop begins
        warm = spsum.tile([P, SQW], F32, tag="sc_a", name="warm")
        for _ in range(WARMUP):
            nc.tensor.matmul(
                warm, kT_sb[:, 0, 0:P], qT_sb[:, 0, 0:SQW], start=True, stop=True
            )

    if True:
        for h in range(HPC):
            outp = [
                opsum.tile([D + 1, SQW], F32, tag=f"o{j}", name=f"outp{j}")
                for j in range(SQB)
            ]
            et_prev = None
            # software pipeline: PE order is QK(t) ... PV(t-1) so the next
            # scores tile is in flight before PV stalls on exp/mask of t-1
            for t in range(SKT + 1):
                if t < SKT:
                    # two independent half-tiles so exp(half a) releases its
                    # psum banks while QK of the other half still runs
                    if SCFULL:
                        sc_f = spsum.tile([P, S], F32, tag="sc_a", name="sc_f")
                        halves = [(0, sc_f)]
                        qk_dsts = [sc_f[:, j * SQW : (j + 1) * SQW] for j in range(SQB)]
                    else:
                        sc_a = spsum.tile([P, HALF], F32, tag="sc_a")
                        sc_b = spsum.tile([P, HALF], F32, tag="sc_b")
                        halves = [(0, sc_a), (1, sc_b)]
                        qk_dsts = [
                            (sc_a, sc_b)[j // 2][:, (j % 2) * SQW : (j % 2 + 1) * SQW]
                            for j in range(SQB)
                        ]
                    kw = kT_sb[:, h, t * P : (t + 1) * P]
                    for j in range(SQB):
                        mm(
                            qk_dsts[j],
                            kw,
                            qT_sb[:, h, j * SQW : (j + 1) * SQW],
                            start=True,
                            stop=True,
                        )
                def emit_pv():
                    tp = t - 1
                    vw = vA_sb[:, h, tp, :]
                    for j in range(SQB):
                        mm(
                            outp[j],
                            vw,
                            et_prev[:, j * SQW : (j + 1) * SQW],
                            start=(tp == 0),
                            stop=(tp == SKT - 1),
                        )

                if t >= 1 and not PVLAST:
                    emit_pv()
                if t < SKT:
                    et = epool.tile([P, S], BF16, tag=f"e{t}")
                    for half, sch in halves:
                        hs = (
                            slice(0, S)
                            if SCFULL
                            else slice(half * HALF, (half + 1) * HALF)
                        )
                        if VARIANT == "noact":
                            # DVE-only writer: times PE+DVE pace without ACT
                            nc.vector.tensor_mul(
                                et[:, hs], mask_sb[:, t, hs], mask_sb[:, t, hs]
                            )
                        else:
                            nc.scalar.activation(et[:, hs], sch, Exp, scale=0.125)
                        if VARIANT == "full" and not BIGMASK:
                            nc.vector.tensor_mul(
                                et[:, hs], et[:, hs], mask_sb[:, t, hs]
                            )
                    if VARIANT == "full" and BIGMASK:
                        nc.vector.tensor_mul(et, et, mask_sb[:, t, :])
                if t >= 1 and PVLAST:
                    emit_pv()
                if t < SKT:
                    et_prev = et

            for j in range(SQB):
                if HOSTNORM:
                    # evict numerator+denominator to SBUF (PSUM is not DMA-
                    # readable), DMA out; host divides during unshard
                    on = ostage.tile([D + 1, SQW], F32, tag="on", name="on")
                    if EVICT_ACT:
                        nc.scalar.copy(on, outp[j])
                    else:
                        nc.vector.tensor_copy(on, outp[j])
                    nc.sync.dma_start(
                        outT_d[h, :, j * SQW : (j + 1) * SQW], on
                    )
                    continue
                if EVICT:
                    # evict the accumulator to SBUF immediately so the PSUM
                    # bank frees for the next head's PV
                    src = ostage.tile([D + 1, SQW], F32, tag="ocp", name="ocp")
                    nc.vector.tensor_copy(src, outp[j])
                else:
                    src = outp[j]
                rec = small.tile([1, SQW], F32, tag="rec")
                nc.vector.reciprocal(rec, src[D : D + 1, :])
                recd = dpool.tile([1, SQW], F32, tag="recd")
                nc.sync.dma_start(recd, rec)
                recb = small.tile([D, SQW], F32, tag="recb")
                nc.sync.dma_start(recb, recd.to_broadcast((D, SQW)))
                on = ostage.tile([D, SQW], F32, tag="on")
                nc.vector.tensor_mul(on, src[0:D, :], recb)
                nc.sync.dma_start(outT_d[h, :, j * SQW : (j + 1) * SQW], on)


def _ap_key(ap):
    return (ap.memref, ap.offset, str(ap.ap), str(ap.dtype))


def _dedup_ldweights(nc):
    """Remove back-to-back PE weight reloads of the identical stationary
    operand. Tile lowering emits one Ldweights per Matmult; QK reuses one
    [64,128] weight for 4 matmuls and PV one [128,65] for 4, so 3/4 of the
    loads are redundant PE issue time. Waits on a removed Ldweights move to
    its (adjacent) Matmult."""
    removed = 0
    for bb in nc.m.functions[0].blocks:
        insts = bb.instructions
        new_list = []
        last_key = None
        pending_waits = []
        for ins in insts:
            if str(ins.engine) != "EngineType.PE":
                new_list.append(ins)
                continue
            if ins.opcode == "Ldweights":
                key = (_ap_key(ins.ins[0]), str(ins.tile_position))
                si = ins.sync_info
                if key == last_key and not (si and si.on_update):
                    if si and si.on_wait:
                        pending_waits.extend(si.on_wait)
                    removed += 1
                    continue
                last_key = key
                new_list.append(ins)
            elif ins.opcode == "Matmult":
                if pending_waits:
                    si = ins.sync_info
                    import bass_rust

                    old_waits = list(si.on_wait) if si else []
                    old_upd = list(si.on_update) if si else []
                    ins.sync_info = bass_rust.SyncInfo(
                        on_wait=old_waits + pending_waits,
                        on_update=old_upd,
                    )
                    pending_waits = []
                new_list.append(ins)
            else:
                # any other PE instruction: conservatively forget weight state
                last_key = None
                new_list.append(ins)
        assert not pending_waits
        insts[:] = new_list
    return removed


_NC_CACHE = {}


def _build(loop_n=0, hoist=False):
    key = (
        loop_n, hoist, VARIANT, CHAIN_PE, DEDUP_LDW, EVICT, WARMUP, HOSTNORM,
        OBUF, BIGMASK, SCFULL, PVLAST, EVICT_ACT,
    )
    if key in _NC_CACHE:
        return _NC_CACHE[key]
    nc = bacc.Bacc(
        "TRN2", target_bir_lowering=False, debug=False, num_devices=NCORES
    )
    qT_d = nc.dram_tensor("qT", [HPC, D, S], BF16, kind="ExternalInput").ap()
    kT_d = nc.dram_tensor("kT", [HPC, D, S], BF16, kind="ExternalInput").ap()
    vA_d = nc.dram_tensor("vA", [HPC, S, D + 1], BF16, kind="ExternalInput").ap()
    mT_d = nc.dram_tensor("mT", [S, S], BF16, kind="ExternalInput").ap()
    od = D + 1 if HOSTNORM else D
    outT_d = nc.dram_tensor("outT", [HPC, od, S], F32, kind="ExternalOutput").ap()
    with tile.TileContext(nc) as tc:
        _emit(tc, qT_d, kT_d, vA_d, mT_d, outT_d, loop_n=loop_n, hoist=hoist)
    if DEDUP_LDW:
        _dedup_ldweights(nc)
    nc.compile()
    _NC_CACHE[key] = nc
    return nc


def _host_prep(q, k, v, mask):
    qf = np.asarray(q, np.float32).reshape(B * H, S, D)
    kf = np.asarray(k, np.float32).reshape(B * H, S, D)
    vf = np.asarray(v, np.float32).reshape(B * H, S, D)
    keepT = np.ascontiguousarray(
        (1.0 - np.asarray(mask[0, 0], np.float32)).T.astype(NPBF16)
    )
    in_maps = []
    for c in range(NCORES):
        sl = slice(c * HPC, (c + 1) * HPC)
        qT = np.ascontiguousarray(qf[sl].transpose(0, 2, 1)).astype(NPBF16)
        kT = np.ascontiguousarray(kf[sl].transpose(0, 2, 1)).astype(NPBF16)
        vA = np.concatenate(
            [vf[sl], np.ones((HPC, S, 1), np.float32)], axis=2
        ).astype(NPBF16)
        in_maps.append({"qT": qT, "kT": kT, "vA": vA, "mT": keepT})
    return in_maps


def _gather(results):
    outs = []
    for c in range(NCORES):
        o = results[c]["outT"]
        if HOSTNORM:
            o = o[:, :D, :] / o[:, D : D + 1, :]
        outs.append(o.transpose(0, 2, 1))
    return np.ascontiguousarray(
        np.concatenate(outs, axis=0).reshape(B, H, S, D)
    ).astype(np.float32)


def kernel(q, k, v, mask):
    global LAST_EXEC_TIME_NS, LAST_RESULTS
    nc = _build()
    in_maps = _host_prep(q, k, v, mask)
    trace = os.environ.get("ATTN_TRACE", "0") == "1"
    res = run_bass_kernel_spmd(
        nc, in_maps, core_ids=list(range(NCORES)), trace=trace
    )
    LAST_EXEC_TIME_NS = res.exec_time_ns
    LAST_RESULTS = res
    return _gather(res.results)



# revision 2
# speedup vs baseline: 5.5104x; 5.5104x over previous
"""Masked softmax attention (B=2,H=16,S=2048,D=64) on 8 trn2 NeuronCores.

Sharding: B*H=32 head-slices, 4 heads per core (pure data parallel),
mask replicated. Per head the device computes scores TRANSPOSED
(S_T[k,q] = K @ Q^T, contract d=64) so softmax and attn@V need no
on-chip transposes:

  QK:   sc[k,q] = kT_tile.T @ qT      (PE, bf16, 16 k-tiles x 2 halves)
  exp:  E[k,q]  = exp(sc/8)           (ACT f32->bf16; some halves on DVE
                                       via a calibrated bf16 exp bit-hack
                                       + quadratic mantissa correction)
  mask: E &= keep16[k,q]              (DVE bitwise AND on u16 views;
                                       0xFFFF keep / 0x0000 drop. DVE-
                                       hacked halves fold the mask into
                                       the hack's bias term instead)
  PV:   outT[m,q] += vA_tile.T @ E    (PE, vA = [V | ones] so row m=64
                                       accumulates the softmax denom)

Cores return numerator+denominator; host divides and transposes.

Engine budget per core (4 heads x 16 k-tiles x [128,2048] tiles):
PE 109us (QK+PV bf16), ACT ~1.04us per [128,1024] exp half, DVE
~3.0us per hacked half + 0.6us per AND + evictions. DVE_T assigns
which sc_b halves go to DVE to balance ACT/DVE under the PE roofline.
"""

import os
import sys
from contextlib import ExitStack

import numpy as np

for _p in ("/opt/trn_rl_repo",):
    if _p not in sys.path:
        sys.path.insert(0, _p)

import ml_dtypes  # noqa: E402

import concourse.bass as bass  # noqa: E402
import concourse.mybir as mybir  # noqa: E402
import concourse.tile as tile  # noqa: E402
from concourse import bacc  # noqa: E402
from concourse.bass_utils import run_bass_kernel_spmd  # noqa: E402

B, H, S, D = 2, 16, 2048, 64
NCORES = 8
HPC = (B * H) // NCORES  # heads per core
P = 128
SKT = S // P  # 16 k-tiles
HALF = S // 2  # 1024
CH = 512  # matmul chunk (one psum bank of f32)

BF16 = mybir.dt.bfloat16
F32 = mybir.dt.float32
U16 = mybir.dt.uint16
I16 = mybir.dt.int16
NPBF16 = ml_dtypes.bfloat16

# exp bit-hack constants (bf16 bits of exp(s*0.125) ~= s*C16 + CB):
C16 = float(0.125 * np.log2(np.e) * 128)  # 23.0831
CB = 16256.5
CB_DROP = -31000.0  # masked: bits ~ -31k -> bf16 -1e-37 ~ 0
# quadratic mantissa correction w = QC0 + QC2*((bits&127) - QM)^2,
# E = E_hack * w;  g(f)=2^f/(1+f) fitted over f=[0,1)
_f = np.linspace(0, 1, 1025)[:-1]
_g = 2.0**_f / (1.0 + _f)
_A = np.stack([np.ones_like(_f), _f, _f * _f], 1)
_c0, _c1, _c2 = np.linalg.lstsq(_A, _g, rcond=None)[0]
QM = float(-_c1 / (2 * _c2) * 128.0)  # center in bit units
QC2 = float(_c2 / (128.0 * 128.0))
QC0 = float(_c0 - _c1 * _c1 / (4 * _c2))

# which k-tile indices get their sc_b half exp'd on DVE (same for all
# heads; per-head halves on DVE = len(DVE_T))
DVE_T = ()

LAST_EXEC_TIME_NS = None
LAST_RESULTS = None


def _emit(tc, ins, outT_d, loop_n=0, hoist=False):
    nc = tc.nc
    with ExitStack() as ctx:
        const = ctx.enter_context(tc.tile_pool(name="const", bufs=1))
        epool = ctx.enter_context(tc.tile_pool(name="epool", bufs=3))
        tpool = ctx.enter_context(tc.tile_pool(name="tpool", bufs=2))
        ostage = ctx.enter_context(tc.tile_pool(name="ostage", bufs=8))
        spsum = ctx.enter_context(tc.tile_pool(name="spsum", bufs=1, space="PSUM"))
        opsum = ctx.enter_context(tc.tile_pool(name="opsum", bufs=1, space="PSUM"))

        qT_d, kT_d, vA_d, mand_d, bias_d = ins
        nd = len(DVE_T)

        # resident inputs, DMA'd in consumption order
        qT_sb = const.tile([D, HPC, S], BF16)
        kT_sb = const.tile([D, HPC, S], BF16)
        vA_sb = const.tile([P, HPC, SKT, D + 1], BF16)
        mand_sb = const.tile([P, SKT, S], U16)
        bias_sb = const.tile([P, nd, HALF], BF16) if nd else None

        nc.sync.dma_start(kT_sb[:, 0, :], kT_d[0])
        nc.sync.dma_start(qT_sb[:, 0, :], qT_d[0])
        nc.sync.dma_start(
            vA_sb[:, 0, :, :], vA_d[0].rearrange("(c p) e -> p c e", p=P)
        )
        di = {t: i for i, t in enumerate(DVE_T)}
        for t in range(SKT):
            if t in di:
                nc.sync.dma_start(bias_sb[:, di[t], :], bias_d[di[t]])
                nc.sync.dma_start(
                    mand_sb[:, t, 0:HALF], mand_d[t * P : (t + 1) * P, 0:HALF]
                )
            else:
                nc.sync.dma_start(mand_sb[:, t, :], mand_d[t * P : (t + 1) * P, :])
        for h in range(1, HPC):
            nc.sync.dma_start(qT_sb[:, h, :], qT_d[h])
            nc.sync.dma_start(kT_sb[:, h, :], kT_d[h])
            nc.sync.dma_start(
                vA_sb[:, h, :, :], vA_d[h].rearrange("(c p) e -> p c e", p=P)
            )

        env = dict(locals())
        if loop_n and hoist:
            with tc.For_i(0, loop_n, 1, hint_engines=(mybir.EngineType.PE,)):
                _compute(tc, env)
            return
        _compute(tc, env)


def _compute(tc, env):
    nc = tc.nc
    Exp = mybir.ActivationFunctionType.Exp
    AND = mybir.AluOpType.bitwise_and
    qT_sb = env["qT_sb"]
    kT_sb = env["kT_sb"]
    vA_sb = env["vA_sb"]
    mand_sb = env["mand_sb"]
    bias_sb = env["bias_sb"]
    epool = env["epool"]
    tpool = env["tpool"]
    ostage = env["ostage"]
    spsum = env["spsum"]
    opsum = env["opsum"]
    outT_d = env["outT_d"]
    di = {t: i for i, t in enumerate(DVE_T)}

    for h in range(HPC):
        outp = [
            opsum.tile([D + 1, CH], F32, tag=f"o{j}", name=f"outp{j}")
            for j in range(4)
        ]
        et_prev = None
        for t in range(SKT + 1):
            if t < SKT:
                sc_a = spsum.tile([P, HALF], F32, tag="sc_a")
                sc_b = spsum.tile([P, HALF], F32, tag="sc_b")
                kw = kT_sb[:, h, t * P : (t + 1) * P]
                for j in range(4):
                    dst = (sc_a, sc_b)[j // 2][:, (j % 2) * CH : (j % 2 + 1) * CH]
                    nc.tensor.matmul(
                        dst,
                        kw,
                        qT_sb[:, h, j * CH : (j + 1) * CH],
                        start=True,
                        stop=True,
                    )

            def emit_pv():
                tp = t - 1
                vw = vA_sb[:, h, tp, :]
                for j in range(4):
                    nc.tensor.matmul(
                        outp[j],
                        vw,
                        et_prev[:, j * CH : (j + 1) * CH],
                        start=(tp == 0),
                        stop=(tp == SKT - 1),
                    )

            if t >= 1:
                emit_pv()

            if t < SKT:
                et = epool.tile([P, S], BF16, tag="e")
                # first half: always ACT
                nc.scalar.activation(et[:, 0:HALF], sc_a, Exp, scale=0.125)
                if t in di:
                    # DVE bit-hack exp on sc_b (mask folded into bias)
                    eb16 = et[:, HALF:S].bitcast(I16)
                    nc.vector.scalar_tensor_tensor(
                        eb16, sc_b, C16, bias_sb[:, di[t], :],
                        mybir.AluOpType.mult, mybir.AluOpType.add,
                    )
                    dm = tpool.tile([P, HALF], BF16, tag="dm")
                    nc.vector.tensor_scalar(
                        dm, et[:, HALF:S].bitcast(U16), 127, QM,
                        AND, mybir.AluOpType.subtract,
                    )
                    sq = tpool.tile([P, HALF], BF16, tag="sq")
                    nc.vector.tensor_tensor(sq, dm, dm, mybir.AluOpType.mult)
                    w = tpool.tile([P, HALF], BF16, tag="w")
                    nc.vector.tensor_scalar(
                        w, sq, QC2, QC0,
                        mybir.AluOpType.mult, mybir.AluOpType.add,
                    )
                    nc.vector.tensor_tensor(
                        et[:, HALF:S], et[:, HALF:S], w, mybir.AluOpType.mult
                    )
                    # AND-mask only the ACT half
                    nc.vector.tensor_tensor(
                        et[:, 0:HALF].bitcast(U16),
                        et[:, 0:HALF].bitcast(U16),
                        mand_sb[:, t, 0:HALF],
                        AND,
                    )
                else:
                    nc.scalar.activation(
                        et[:, HALF:S], sc_b, Exp, scale=0.125
                    )
                    nc.vector.tensor_tensor(
                        et.bitcast(U16), et.bitcast(U16), mand_sb[:, t, :], AND
                    )
                et_prev = et

        for j in range(4):
            on = ostage.tile([D + 1, CH], F32, tag="on", name="on")
            nc.vector.tensor_copy(on, outp[j])
            nc.sync.dma_start(outT_d[h, :, j * CH : (j + 1) * CH], on)


_NC_CACHE = {}


def _build(loop_n=0, hoist=False):
    key = (loop_n, hoist, DVE_T)
    if key in _NC_CACHE:
        return _NC_CACHE[key]
    nd = len(DVE_T)
    nc = bacc.Bacc(
        "TRN2", target_bir_lowering=False, debug=False, num_devices=NCORES
    )
    qT_d = nc.dram_tensor("qT", [HPC, D, S], BF16, kind="ExternalInput").ap()
    kT_d = nc.dram_tensor("kT", [HPC, D, S], BF16, kind="ExternalInput").ap()
    vA_d = nc.dram_tensor("vA", [HPC, S, D + 1], BF16, kind="ExternalInput").ap()
    mand_d = nc.dram_tensor("mand", [S, S], U16, kind="ExternalInput").ap()
    bias_d = nc.dram_tensor(
        "biasT", [max(nd, 1), P, HALF], BF16, kind="ExternalInput"
    ).ap()
    outT_d = nc.dram_tensor(
        "outT", [HPC, D + 1, S], F32, kind="ExternalOutput"
    ).ap()
    with tile.TileContext(nc) as tc:
        _emit(
            tc, (qT_d, kT_d, vA_d, mand_d, bias_d), outT_d,
            loop_n=loop_n, hoist=hoist,
        )
    nc.compile()
    _NC_CACHE[key] = nc
    return nc


def _host_prep(q, k, v, mask):
    qf = np.asarray(q, np.float32).reshape(B * H, S, D)
    kf = np.asarray(k, np.float32).reshape(B * H, S, D)
    vf = np.asarray(v, np.float32).reshape(B * H, S, D)
    nd = len(DVE_T)
    # transposed keep mask [k, q]: 0xFFFF keep / 0x0000 drop
    dropT = np.ascontiguousarray(np.asarray(mask[0, 0], np.int32).T)
    keep16 = np.where(dropT != 0, 0, 0xFFFF).astype(np.uint16)
    # hack bias per DVE tile: [nd, 128, 1024]
    if nd:
        biasT = np.empty((nd, P, HALF), np.float32)
        for i, t in enumerate(DVE_T):
            drop_slice = dropT[t * P : (t + 1) * P, HALF:S]
            biasT[i] = np.where(drop_slice != 0, CB_DROP, CB)
        biasT = biasT.astype(NPBF16)
    else:
        biasT = np.zeros((1, P, HALF), NPBF16)
    in_maps = []
    for c in range(NCORES):
        sl = slice(c * HPC, (c + 1) * HPC)
        qT = np.ascontiguousarray(qf[sl].transpose(0, 2, 1)).astype(NPBF16)
        kT = np.ascontiguousarray(kf[sl].transpose(0, 2, 1)).astype(NPBF16)
        vA = np.concatenate(
            [vf[sl], np.ones((HPC, S, 1), np.float32)], axis=2
        ).astype(NPBF16)
        in_maps.append(
            {"qT": qT, "kT": kT, "vA": vA, "mand": keep16, "biasT": biasT}
        )
    return in_maps


def _gather(results):
    outs = []
    for c in range(NCORES):
        o = results[c]["outT"]
        o = o[:, :D, :] / o[:, D : D + 1, :]
        outs.append(o.transpose(0, 2, 1))
    return np.ascontiguousarray(
        np.concatenate(outs, axis=0).reshape(B, H, S, D)
    ).astype(np.float32)


def kernel(q, k, v, mask):
    global LAST_EXEC_TIME_NS, LAST_RESULTS
    nc = _build()
    in_maps = _host_prep(q, k, v, mask)
    trace = os.environ.get("ATTN_TRACE", "0") == "1"
    res = run_bass_kernel_spmd(
        nc, in_maps, core_ids=list(range(NCORES)), trace=trace
    )
    LAST_EXEC_TIME_NS = res.exec_time_ns
    LAST_RESULTS = res
    return _gather(res.results)


# revision 3
# speedup vs baseline: 6.4675x; 1.1737x over previous
"""Masked softmax attention (B=2,H=16,S=2048,D=64) on 8 trn2 NeuronCores.

Sharding: B*H=32 head-slices, 4 heads per core (pure data parallel),
mask replicated. Per head the device computes scores TRANSPOSED
(S_T[k,q] = K @ Q^T, contract d=64) so softmax and attn@V need no
on-chip transposes:

  QK:   sc[k,q] = kT_tile.T @ qT      (PE, bf16, 16 k-tiles x 2 halves)
  exp:  E[k,q]  = exp(sc/8)           (ACT f32->bf16; some halves on DVE
                                       via a calibrated bf16 exp bit-hack
                                       + quadratic mantissa correction)
  mask: E &= keep16[k,q]              (DVE bitwise AND on u16 views;
                                       0xFFFF keep / 0x0000 drop. DVE-
                                       hacked halves fold the mask into
                                       the hack's bias term instead)
  PV:   outT[m,q] += vA_tile.T @ E    (PE, vA = [V | ones] so row m=64
                                       accumulates the softmax denom)

Cores return numerator+denominator; host divides and transposes.

Engine budget per core (4 heads x 16 k-tiles x [128,2048] tiles):
PE 109us (QK+PV bf16), ACT ~1.04us per [128,1024] exp half, DVE
~3.0us per hacked half + 0.6us per AND + evictions. DVE_T assigns
which sc_b halves go to DVE to balance ACT/DVE under the PE roofline.
"""

import os
import sys
from contextlib import ExitStack

import numpy as np

for _p in ("/opt/trn_rl_repo",):
    if _p not in sys.path:
        sys.path.insert(0, _p)

import ml_dtypes  # noqa: E402

import concourse.bass as bass  # noqa: E402
import concourse.mybir as mybir  # noqa: E402
import concourse.tile as tile  # noqa: E402
from concourse import bacc  # noqa: E402
from concourse.bass_utils import run_bass_kernel_spmd  # noqa: E402

B, H, S, D = 2, 16, 2048, 64
NCORES = 8
HPC = (B * H) // NCORES  # heads per core
P = 128
SKT = S // P  # 16 k-tiles
HALF = S // 2  # 1024
CH = 512  # matmul chunk (one psum bank of f32)

BF16 = mybir.dt.bfloat16
F32 = mybir.dt.float32
U16 = mybir.dt.uint16
I16 = mybir.dt.int16
NPBF16 = ml_dtypes.bfloat16

# exp bit-hack constants (bf16 bits of exp(s*0.125) ~= s*C16 + CB):
C16 = float(0.125 * np.log2(np.e) * 128)  # 23.0831
CB = 16256.5
CB_DROP = -31000.0  # masked: bits ~ -31k -> bf16 -1e-37 ~ 0
# quadratic mantissa correction w = QC0 + QC2*((bits&127) - QM)^2,
# E = E_hack * w;  g(f)=2^f/(1+f) fitted over f=[0,1)
_f = np.linspace(0, 1, 1025)[:-1]
_g = 2.0**_f / (1.0 + _f)
_A = np.stack([np.ones_like(_f), _f, _f * _f], 1)
_c0, _c1, _c2 = np.linalg.lstsq(_A, _g, rcond=None)[0]
QM = float(-_c1 / (2 * _c2) * 128.0)  # center in bit units
QC2 = float(_c2 / (128.0 * 128.0))
QC0 = float(_c0 - _c1 * _c1 / (4 * _c2))

# which k-tile indices get their sc_b half exp'd on DVE (same for all
# heads; per-head halves on DVE = len(DVE_T))
DVE_T = ()

LAST_EXEC_TIME_NS = None
LAST_RESULTS = None


def _emit(tc, ins, outT_d, loop_n=0, hoist=False):
    nc = tc.nc
    with ExitStack() as ctx:
        const = ctx.enter_context(tc.tile_pool(name="const", bufs=1))
        epool = ctx.enter_context(tc.tile_pool(name="epool", bufs=3))
        tpool = ctx.enter_context(tc.tile_pool(name="tpool", bufs=2))
        ostage = ctx.enter_context(tc.tile_pool(name="ostage", bufs=8))
        spsum = ctx.enter_context(tc.tile_pool(name="spsum", bufs=1, space="PSUM"))
        opsum = ctx.enter_context(tc.tile_pool(name="opsum", bufs=1, space="PSUM"))

        qT_d, kT_d, vA_d, mand_d, bias_d = ins
        nd = len(DVE_T)

        # resident inputs, DMA'd in consumption order
        qT_sb = const.tile([D, HPC, S], BF16)
        kT_sb = const.tile([D, HPC, S], BF16)
        vA_sb = const.tile([P, HPC, SKT, D + 1], BF16)
        mand_sb = const.tile([P, SKT, S], U16)
        bias_sb = const.tile([P, nd, HALF], BF16) if nd else None

        nc.sync.dma_start(kT_sb[:, 0, :], kT_d[0])
        nc.sync.dma_start(qT_sb[:, 0, :], qT_d[0])
        nc.sync.dma_start(
            vA_sb[:, 0, :, :], vA_d[0].rearrange("(c p) e -> p c e", p=P)
        )
        di = {t: i for i, t in enumerate(DVE_T)}
        for t in range(SKT):
            if t in di:
                nc.sync.dma_start(bias_sb[:, di[t], :], bias_d[di[t]])
                nc.sync.dma_start(
                    mand_sb[:, t, 0:HALF], mand_d[t * P : (t + 1) * P, 0:HALF]
                )
            else:
                nc.sync.dma_start(mand_sb[:, t, :], mand_d[t * P : (t + 1) * P, :])
        for h in range(1, HPC):
            nc.sync.dma_start(qT_sb[:, h, :], qT_d[h])
            nc.sync.dma_start(kT_sb[:, h, :], kT_d[h])
            nc.sync.dma_start(
                vA_sb[:, h, :, :], vA_d[h].rearrange("(c p) e -> p c e", p=P)
            )

        env = dict(locals())
        if loop_n and hoist:
            with tc.For_i(0, loop_n, 1, hint_engines=(mybir.EngineType.PE,)):
                _compute(tc, env)
            return
        _compute(tc, env)


def _compute(tc, env):
    nc = tc.nc
    Exp = mybir.ActivationFunctionType.Exp
    AND = mybir.AluOpType.bitwise_and
    qT_sb = env["qT_sb"]
    kT_sb = env["kT_sb"]
    vA_sb = env["vA_sb"]
    mand_sb = env["mand_sb"]
    bias_sb = env["bias_sb"]
    epool = env["epool"]
    tpool = env["tpool"]
    ostage = env["ostage"]
    spsum = env["spsum"]
    opsum = env["opsum"]
    outT_d = env["outT_d"]
    di = {t: i for i, t in enumerate(DVE_T)}

    PVD = 2  # PV trails QK by PVD tiles so E-production latency is hidden
    for h in range(HPC):
        outp = [
            opsum.tile([D + 1, CH], F32, tag=f"o{j}", name=f"outp{j}")
            for j in range(4)
        ]
        ehist = {}
        for t in range(SKT + PVD):
            if t < SKT:
                sc_a = spsum.tile([P, HALF], F32, tag="sc_a")
                sc_b = spsum.tile([P, HALF], F32, tag="sc_b")
                kw = kT_sb[:, h, t * P : (t + 1) * P]
                for j in range(4):
                    dst = (sc_a, sc_b)[j // 2][:, (j % 2) * CH : (j % 2 + 1) * CH]
                    nc.tensor.matmul(
                        dst,
                        kw,
                        qT_sb[:, h, j * CH : (j + 1) * CH],
                        start=True,
                        stop=True,
                    )

            if t >= PVD:
                tp = t - PVD
                vw = vA_sb[:, h, tp, :]
                epv = ehist.pop(tp)
                for j in range(4):
                    nc.tensor.matmul(
                        outp[j],
                        vw,
                        epv[:, j * CH : (j + 1) * CH],
                        start=(tp == 0),
                        stop=(tp == SKT - 1),
                    )

            if t < SKT:
                et = epool.tile([P, S], BF16, tag="e")
                # first half: always ACT, then mask-AND the half right away
                nc.scalar.activation(et[:, 0:HALF], sc_a, Exp, scale=0.125)
                nc.vector.tensor_tensor(
                    et[:, 0:HALF].bitcast(U16),
                    et[:, 0:HALF].bitcast(U16),
                    mand_sb[:, t, 0:HALF],
                    AND,
                )
                if t in di:
                    # DVE bit-hack exp on sc_b (mask folded into bias)
                    eb16 = et[:, HALF:S].bitcast(I16)
                    nc.vector.scalar_tensor_tensor(
                        eb16, sc_b, C16, bias_sb[:, di[t], :],
                        mybir.AluOpType.mult, mybir.AluOpType.add,
                    )
                    dm = tpool.tile([P, HALF], BF16, tag="dm")
                    nc.vector.tensor_scalar(
                        dm, et[:, HALF:S].bitcast(U16), 127, QM,
                        AND, mybir.AluOpType.subtract,
                    )
                    sq = tpool.tile([P, HALF], BF16, tag="sq")
                    nc.vector.tensor_tensor(sq, dm, dm, mybir.AluOpType.mult)
                    w = tpool.tile([P, HALF], BF16, tag="w")
                    nc.vector.tensor_scalar(
                        w, sq, QC2, QC0,
                        mybir.AluOpType.mult, mybir.AluOpType.add,
                    )
                    nc.vector.tensor_tensor(
                        et[:, HALF:S], et[:, HALF:S], w, mybir.AluOpType.mult
                    )
                else:
                    nc.scalar.activation(
                        et[:, HALF:S], sc_b, Exp, scale=0.125
                    )
                    nc.vector.tensor_tensor(
                        et[:, HALF:S].bitcast(U16),
                        et[:, HALF:S].bitcast(U16),
                        mand_sb[:, t, HALF:S],
                        AND,
                    )
                ehist[t] = et

        for j in range(4):
            on = ostage.tile([D + 1, CH], F32, tag="on", name="on")
            nc.vector.tensor_copy(on, outp[j])
            nc.sync.dma_start(outT_d[h, :, j * CH : (j + 1) * CH], on)


_NC_CACHE = {}


def _build(loop_n=0, hoist=False):
    key = (loop_n, hoist, DVE_T)
    if key in _NC_CACHE:
        return _NC_CACHE[key]
    nd = len(DVE_T)
    nc = bacc.Bacc(
        "TRN2", target_bir_lowering=False, debug=False, num_devices=NCORES
    )
    qT_d = nc.dram_tensor("qT", [HPC, D, S], BF16, kind="ExternalInput").ap()
    kT_d = nc.dram_tensor("kT", [HPC, D, S], BF16, kind="ExternalInput").ap()
    vA_d = nc.dram_tensor("vA", [HPC, S, D + 1], BF16, kind="ExternalInput").ap()
    mand_d = nc.dram_tensor("mand", [S, S], U16, kind="ExternalInput").ap()
    bias_d = nc.dram_tensor(
        "biasT", [max(nd, 1), P, HALF], BF16, kind="ExternalInput"
    ).ap()
    outT_d = nc.dram_tensor(
        "outT", [HPC, D + 1, S], F32, kind="ExternalOutput"
    ).ap()
    with tile.TileContext(nc) as tc:
        _emit(
            tc, (qT_d, kT_d, vA_d, mand_d, bias_d), outT_d,
            loop_n=loop_n, hoist=hoist,
        )
    nc.compile()
    _NC_CACHE[key] = nc
    return nc


def _host_prep(q, k, v, mask):
    qf = np.asarray(q, np.float32).reshape(B * H, S, D)
    kf = np.asarray(k, np.float32).reshape(B * H, S, D)
    vf = np.asarray(v, np.float32).reshape(B * H, S, D)
    nd = len(DVE_T)
    # transposed keep mask [k, q]: 0xFFFF keep / 0x0000 drop
    dropT = np.ascontiguousarray(np.asarray(mask[0, 0], np.int32).T)
    keep16 = np.where(dropT != 0, 0, 0xFFFF).astype(np.uint16)
    # hack bias per DVE tile: [nd, 128, 1024]
    if nd:
        biasT = np.empty((nd, P, HALF), np.float32)
        for i, t in enumerate(DVE_T):
            drop_slice = dropT[t * P : (t + 1) * P, HALF:S]
            biasT[i] = np.where(drop_slice != 0, CB_DROP, CB)
        biasT = biasT.astype(NPBF16)
    else:
        biasT = np.zeros((1, P, HALF), NPBF16)
    in_maps = []
    for c in range(NCORES):
        sl = slice(c * HPC, (c + 1) * HPC)
        qT = np.ascontiguousarray(qf[sl].transpose(0, 2, 1)).astype(NPBF16)
        kT = np.ascontiguousarray(kf[sl].transpose(0, 2, 1)).astype(NPBF16)
        vA = np.concatenate(
            [vf[sl], np.ones((HPC, S, 1), np.float32)], axis=2
        ).astype(NPBF16)
        in_maps.append(
            {"qT": qT, "kT": kT, "vA": vA, "mand": keep16, "biasT": biasT}
        )
    return in_maps


def _gather(results):
    outs = []
    for c in range(NCORES):
        o = results[c]["outT"]
        o = o[:, :D, :] / o[:, D : D + 1, :]
        outs.append(o.transpose(0, 2, 1))
    return np.ascontiguousarray(
        np.concatenate(outs, axis=0).reshape(B, H, S, D)
    ).astype(np.float32)


def kernel(q, k, v, mask):
    global LAST_EXEC_TIME_NS, LAST_RESULTS
    nc = _build()
    in_maps = _host_prep(q, k, v, mask)
    trace = os.environ.get("ATTN_TRACE", "0") == "1"
    res = run_bass_kernel_spmd(
        nc, in_maps, core_ids=list(range(NCORES)), trace=trace
    )
    LAST_EXEC_TIME_NS = res.exec_time_ns
    LAST_RESULTS = res
    return _gather(res.results)


# revision 6
# speedup vs baseline: 9.6739x; 1.4958x over previous
"""Masked softmax attention (B=2,H=16,S=2048,D=64) on 8 trn2 NeuronCores.

Sharding: B*H=32 head-slices, 4 heads per core (pure data parallel),
mask replicated. Per head the device computes scores TRANSPOSED
(S_T[k,q] = K @ Q^T, contract d=64) so softmax and attn@V need no
on-chip transposes:

  QK:   sc[k,q] = kT_tile.T @ qT      (PE, bf16, 16 k-tiles x 2 halves)
  exp:  E[k,q]  = exp(sc/8)           (ACT f32->bf16; some halves on DVE
                                       via a calibrated bf16 exp bit-hack
                                       + quadratic mantissa correction)
  mask: E &= keep16[k,q]              (DVE bitwise AND on u16 views;
                                       0xFFFF keep / 0x0000 drop. DVE-
                                       hacked halves fold the mask into
                                       the hack's bias term instead)
  PV:   outT[m,q] += vA_tile.T @ E    (PE, vA = [V | ones] so row m=64
                                       accumulates the softmax denom)

Cores return numerator+denominator; host divides and transposes.

Engine budget per core (4 heads x 16 k-tiles x [128,2048] tiles):
PE 109us (QK+PV bf16), ACT ~1.04us per [128,1024] exp half, DVE
~3.0us per hacked half + 0.6us per AND + evictions. DVE_T assigns
which sc_b halves go to DVE to balance ACT/DVE under the PE roofline.
"""

import os
import sys
from contextlib import ExitStack

import numpy as np

for _p in ("/opt/trn_rl_repo",):
    if _p not in sys.path:
        sys.path.insert(0, _p)

import ml_dtypes  # noqa: E402

import concourse.bass as bass  # noqa: E402
import concourse.mybir as mybir  # noqa: E402
import concourse.tile as tile  # noqa: E402
from concourse import bacc  # noqa: E402
from concourse.bass_utils import run_bass_kernel_spmd  # noqa: E402

B, H, S, D = 2, 16, 2048, 64
NCORES = 8
HPC = (B * H) // NCORES  # heads per core
P = 128
SKT = S // P  # 16 k-tiles
HALF = S // 2  # 1024
CH = 512  # matmul chunk (one psum bank of f32)

BF16 = mybir.dt.bfloat16
F32 = mybir.dt.float32
U16 = mybir.dt.uint16
I16 = mybir.dt.int16
NPBF16 = ml_dtypes.bfloat16

# exp bit-hack constants (bf16 bits of exp(s*0.125) ~= s*C16 + CB):
C16 = float(0.125 * np.log2(np.e) * 128)  # 23.0831
CB = 16256.5
CB_DROP = -31000.0  # masked: bits ~ -31k -> bf16 -1e-37 ~ 0
# quadratic mantissa correction w = QC0 + QC2*((bits&127) - QM)^2,
# E = E_hack * w;  g(f)=2^f/(1+f) fitted over f=[0,1)
_f = np.linspace(0, 1, 1025)[:-1]
_g = 2.0**_f / (1.0 + _f)
_A = np.stack([np.ones_like(_f), _f, _f * _f], 1)
_c0, _c1, _c2 = np.linalg.lstsq(_A, _g, rcond=None)[0]
QM = float(-_c1 / (2 * _c2) * 128.0)  # center in bit units
QC2 = float(_c2 / (128.0 * 128.0))
QC0 = float(_c0 - _c1 * _c1 / (4 * _c2))

# which k-tile indices get their sc_b half exp'd on DVE (same for all
# heads; per-head halves on DVE = len(DVE_T))
DVE_T = ()

LAST_EXEC_TIME_NS = None
LAST_RESULTS = None


def _emit(tc, ins, outT_d, loop_n=0, hoist=False):
    nc = tc.nc
    with ExitStack() as ctx:
        const = ctx.enter_context(tc.tile_pool(name="const", bufs=1))
        epool = ctx.enter_context(tc.tile_pool(name="epool", bufs=3))
        tpool = ctx.enter_context(tc.tile_pool(name="tpool", bufs=2))
        ostage = ctx.enter_context(tc.tile_pool(name="ostage", bufs=8))
        spsum = ctx.enter_context(tc.tile_pool(name="spsum", bufs=1, space="PSUM"))
        opsum = ctx.enter_context(tc.tile_pool(name="opsum", bufs=1, space="PSUM"))

        qT_d, kT_d, vA_d, mand_d, bias_d = ins
        nd = len(DVE_T)

        # resident inputs, DMA'd in consumption order
        # d padded 64->128 (zeros): contract-128 matmuls stream the PE
        # moving port at full rate (2x the contract-64 rate)
        qT_sb = const.tile([P, HPC, S], BF16)
        kT_sb = const.tile([P, HPC, S], BF16)
        vA_sb = const.tile([P, HPC, SKT, D + 1], BF16)
        mand_sb = const.tile([P, SKT, S], U16)
        bias_sb = const.tile([P, nd, HALF], BF16) if nd else None

        nc.sync.dma_start(kT_sb[:, 0, :], kT_d[0])
        nc.sync.dma_start(qT_sb[:, 0, :], qT_d[0])
        nc.sync.dma_start(
            vA_sb[:, 0, :, :], vA_d[0].rearrange("(c p) e -> p c e", p=P)
        )
        di = {t: i for i, t in enumerate(DVE_T)}
        for t in range(SKT):
            if t in di:
                nc.sync.dma_start(bias_sb[:, di[t], :], bias_d[di[t]])
                nc.sync.dma_start(
                    mand_sb[:, t, 0:HALF], mand_d[t * P : (t + 1) * P, 0:HALF]
                )
            else:
                nc.sync.dma_start(mand_sb[:, t, :], mand_d[t * P : (t + 1) * P, :])
        for h in range(1, HPC):
            nc.sync.dma_start(qT_sb[:, h, :], qT_d[h])
            nc.sync.dma_start(kT_sb[:, h, :], kT_d[h])
            nc.sync.dma_start(
                vA_sb[:, h, :, :], vA_d[h].rearrange("(c p) e -> p c e", p=P)
            )

        env = dict(locals())
        if loop_n and hoist:
            with tc.For_i(0, loop_n, 1, hint_engines=(mybir.EngineType.PE,)):
                _compute(tc, env)
            return
        _compute(tc, env)


def _compute(tc, env):
    nc = tc.nc
    Exp = mybir.ActivationFunctionType.Exp
    AND = mybir.AluOpType.bitwise_and
    qT_sb = env["qT_sb"]
    kT_sb = env["kT_sb"]
    vA_sb = env["vA_sb"]
    mand_sb = env["mand_sb"]
    bias_sb = env["bias_sb"]
    epool = env["epool"]
    tpool = env["tpool"]
    ostage = env["ostage"]
    spsum = env["spsum"]
    opsum = env["opsum"]
    outT_d = env["outT_d"]
    di = {t: i for i, t in enumerate(DVE_T)}

    PVD = 2  # PV trails QK by PVD tiles so E-production latency is hidden
    for h in range(HPC):
        outp = [
            opsum.tile([D + 1, CH], F32, tag=f"o{j}", name=f"outp{j}")
            for j in range(4)
        ]
        ehist = {}
        for t in range(SKT + PVD):
            if t < SKT:
                sc_a = spsum.tile([P, HALF], F32, tag="sc_a")
                sc_b = spsum.tile([P, HALF], F32, tag="sc_b")
                kw = kT_sb[:, h, t * P : (t + 1) * P]
                for j in range(4):
                    dst = (sc_a, sc_b)[j // 2][:, (j % 2) * CH : (j % 2 + 1) * CH]
                    nc.tensor.matmul(
                        dst,
                        kw,
                        qT_sb[:, h, j * CH : (j + 1) * CH],
                        start=True,
                        stop=True,
                    )

            if t >= PVD:
                tp = t - PVD
                vw = vA_sb[:, h, tp, :]
                epv = ehist.pop(tp)
                for j in range(4):
                    nc.tensor.matmul(
                        outp[j],
                        vw,
                        epv[:, j * CH : (j + 1) * CH],
                        start=(tp == 0),
                        stop=(tp == SKT - 1),
                    )

            if t < SKT:
                et = epool.tile([P, S], BF16, tag="e")
                # first half: always ACT, then mask-AND the half right away
                nc.scalar.activation(et[:, 0:HALF], sc_a, Exp, scale=0.125)
                nc.vector.tensor_tensor(
                    et[:, 0:HALF].bitcast(U16),
                    et[:, 0:HALF].bitcast(U16),
                    mand_sb[:, t, 0:HALF],
                    AND,
                )
                if t in di:
                    # DVE bit-hack exp on sc_b (mask folded into bias)
                    eb16 = et[:, HALF:S].bitcast(I16)
                    nc.vector.scalar_tensor_tensor(
                        eb16, sc_b, C16, bias_sb[:, di[t], :],
                        mybir.AluOpType.mult, mybir.AluOpType.add,
                    )
                    dm = tpool.tile([P, HALF], BF16, tag="dm")
                    nc.vector.tensor_scalar(
                        dm, et[:, HALF:S].bitcast(U16), 127, QM,
                        AND, mybir.AluOpType.subtract,
                    )
                    sq = tpool.tile([P, HALF], BF16, tag="sq")
                    nc.vector.tensor_tensor(sq, dm, dm, mybir.AluOpType.mult)
                    w = tpool.tile([P, HALF], BF16, tag="w")
                    nc.vector.tensor_scalar(
                        w, sq, QC2, QC0,
                        mybir.AluOpType.mult, mybir.AluOpType.add,
                    )
                    nc.vector.tensor_tensor(
                        et[:, HALF:S], et[:, HALF:S], w, mybir.AluOpType.mult
                    )
                else:
                    nc.scalar.activation(
                        et[:, HALF:S], sc_b, Exp, scale=0.125
                    )
                    nc.vector.tensor_tensor(
                        et[:, HALF:S].bitcast(U16),
                        et[:, HALF:S].bitcast(U16),
                        mand_sb[:, t, HALF:S],
                        AND,
                    )
                ehist[t] = et

        for j in range(4):
            on = ostage.tile([D + 1, CH], F32, tag="on", name="on")
            nc.vector.tensor_copy(on, outp[j])
            nc.sync.dma_start(outT_d[h, :, j * CH : (j + 1) * CH], on)


def _ap_key(ap):
    return (ap.memref, ap.offset, str(ap.ap), str(ap.dtype))


def _dedup_ldweights(nc):
    """Remove back-to-back PE weight reloads of an identical stationary
    operand (QK's 4 chunk matmuls share one [64,128] weight; PV's 4 share
    one [128,65]). Waits on a removed Ldweights move to its Matmult."""
    import bass_rust

    removed = 0
    for bb in nc.m.functions[0].blocks:
        insts = bb.instructions
        new_list = []
        last_key = None
        pending_waits = []
        for ins in insts:
            if str(ins.engine) != "EngineType.PE":
                new_list.append(ins)
                continue
            if ins.opcode == "Ldweights":
                key = (_ap_key(ins.ins[0]), str(ins.tile_position))
                si = ins.sync_info
                if key == last_key and not (si and si.on_update):
                    if si and si.on_wait:
                        pending_waits.extend(si.on_wait)
                    removed += 1
                    continue
                last_key = key
                new_list.append(ins)
            elif ins.opcode == "Matmult":
                if pending_waits:
                    si = ins.sync_info
                    old_waits = list(si.on_wait) if si else []
                    old_upd = list(si.on_update) if si else []
                    ins.sync_info = bass_rust.SyncInfo(
                        on_wait=old_waits + pending_waits,
                        on_update=old_upd,
                    )
                    pending_waits = []
                new_list.append(ins)
            else:
                last_key = None
                new_list.append(ins)
        assert not pending_waits
        insts[:] = new_list
    return removed


_NC_CACHE = {}
DEDUP_LDW = True


def _build(loop_n=0, hoist=False):
    key = (loop_n, hoist, DVE_T, DEDUP_LDW)
    if key in _NC_CACHE:
        return _NC_CACHE[key]
    nd = len(DVE_T)
    nc = bacc.Bacc(
        "TRN2", target_bir_lowering=False, debug=False, num_devices=NCORES
    )
    qT_d = nc.dram_tensor("qT", [HPC, P, S], BF16, kind="ExternalInput").ap()
    kT_d = nc.dram_tensor("kT", [HPC, P, S], BF16, kind="ExternalInput").ap()
    vA_d = nc.dram_tensor("vA", [HPC, S, D + 1], BF16, kind="ExternalInput").ap()
    mand_d = nc.dram_tensor("mand", [S, S], U16, kind="ExternalInput").ap()
    bias_d = nc.dram_tensor(
        "biasT", [max(nd, 1), P, HALF], BF16, kind="ExternalInput"
    ).ap()
    outT_d = nc.dram_tensor(
        "outT", [HPC, D + 1, S], F32, kind="ExternalOutput"
    ).ap()
    with tile.TileContext(nc) as tc:
        _emit(
            tc, (qT_d, kT_d, vA_d, mand_d, bias_d), outT_d,
            loop_n=loop_n, hoist=hoist,
        )
    if DEDUP_LDW:
        _dedup_ldweights(nc)
    nc.compile()
    _NC_CACHE[key] = nc
    return nc


def _host_prep(q, k, v, mask):
    qf = np.asarray(q, np.float32).reshape(B * H, S, D)
    kf = np.asarray(k, np.float32).reshape(B * H, S, D)
    vf = np.asarray(v, np.float32).reshape(B * H, S, D)
    nd = len(DVE_T)
    # transposed keep mask [k, q]: 0xFFFF keep / 0x0000 drop
    dropT = np.ascontiguousarray(np.asarray(mask[0, 0], np.int32).T)
    keep16 = np.where(dropT != 0, 0, 0xFFFF).astype(np.uint16)
    # hack bias per DVE tile: [nd, 128, 1024]
    if nd:
        biasT = np.empty((nd, P, HALF), np.float32)
        for i, t in enumerate(DVE_T):
            drop_slice = dropT[t * P : (t + 1) * P, HALF:S]
            biasT[i] = np.where(drop_slice != 0, CB_DROP, CB)
        biasT = biasT.astype(NPBF16)
    else:
        biasT = np.zeros((1, P, HALF), NPBF16)
    in_maps = []
    for c in range(NCORES):
        sl = slice(c * HPC, (c + 1) * HPC)
        qT = np.zeros((HPC, P, S), np.float32)
        kT = np.zeros((HPC, P, S), np.float32)
        qT[:, :D, :] = qf[sl].transpose(0, 2, 1)
        kT[:, :D, :] = kf[sl].transpose(0, 2, 1)
        qT = qT.astype(NPBF16)
        kT = kT.astype(NPBF16)
        vA = np.concatenate(
            [vf[sl], np.ones((HPC, S, 1), np.float32)], axis=2
        ).astype(NPBF16)
        in_maps.append(
            {"qT": qT, "kT": kT, "vA": vA, "mand": keep16, "biasT": biasT}
        )
    return in_maps


def _gather(results):
    outs = []
    for c in range(NCORES):
        o = results[c]["outT"]
        o = o[:, :D, :] / o[:, D : D + 1, :]
        outs.append(o.transpose(0, 2, 1))
    return np.ascontiguousarray(
        np.concatenate(outs, axis=0).reshape(B, H, S, D)
    ).astype(np.float32)


def kernel(q, k, v, mask):
    global LAST_EXEC_TIME_NS, LAST_RESULTS
    nc = _build()
    in_maps = _host_prep(q, k, v, mask)
    trace = os.environ.get("ATTN_TRACE", "0") == "1"
    res = run_bass_kernel_spmd(
        nc, in_maps, core_ids=list(range(NCORES)), trace=trace
    )
    LAST_EXEC_TIME_NS = res.exec_time_ns
    LAST_RESULTS = res
    return _gather(res.results)
